# revision 25
# baseline (speedup 1.0000x reference)
"""AudioCrossAttention on 8 Trainium2 NeuronCores.

Sharding: data-parallel over batch (B=2) x tensor-parallel over heads
(16 heads -> 4 heads / 256 dims per core).  Core c handles batch c//4 and
head-group c%4.  Each core computes its 4 heads' attention plus the partial
output projection over its 256-dim slice; partials are summed on the host
(the unshard step) and bo added there.

Everything on device flows in transposed layout ([d, s] / [skv, sq]) so no
transposes are ever needed:
  qT[d,sq]  = WqT.T @ xT          (lhsT=WqT [din,256], rhs=visual.T)
  kT[d,skv] = WkT.T @ xT (+bk +L-RoPE emb, fused into the PSUM eviction)
  v[skv,d]  = xT.T @ WvT  (natural layout, ones column appended per head)
  scoresT[skv,sq] = kT_h.T @ qT_h          per head, K=hd=64
  expT = exp(0.125 * scoresT)              (no max-subtract; scores are O(5))
  [outT; denom] = [v_h | 1].T @ expT       (ones column -> row 64 = denom)
  outT /= denom  (reciprocal -> gpsimd partition_broadcast -> DVE mult)
  finalT[e,sq] += WoT_c.T @ outT           (partial over this core's d-slice)

Matmul operands are fp16 (cast on host): 1 col/cycle at 2.4 GHz on the PE
vs 4x slower fp32 and 2x slower f32r; accumulation stays fp32 in PSUM, the
final projection partials leave the chip in fp32.
"""

import sys

if '/opt/trn_rl_repo' not in sys.path:
    sys.path.insert(0, '/opt/trn_rl_repo')

import numpy as np

B = 2
SQ = 2048
SKV = 2048
DIM = 1024
NUM_HEADS = 16
HEAD_DIM = 64
N_CORES = 8
HPC = 4          # heads per core
DSL = 256        # d_out slice per core
CH = 512         # sq chunk width
NCH = SQ // CH   # 4
KT = DIM // 128  # 8  d_in k-tiles
ST = SKV // 128  # 16 skv tiles
SCALE = HEAD_DIM ** -0.5

_CACHE = {}


def _build():
    import concourse.bacc as bacc
    import concourse.mybir as mybir
    from concourse import tile

    F32 = mybir.dt.float32
    F16 = mybir.dt.float16
    AF = mybir.ActivationFunctionType
    ALU = mybir.AluOpType

    nc = bacc.Bacc("TRN2", target_bir_lowering=False, debug=False,
                   num_devices=N_CORES)

    xq = nc.dram_tensor("xq", [DIM, SQ], F16, kind="ExternalInput")
    xa = nc.dram_tensor("xa", [DIM, SKV], F16, kind="ExternalInput")
    wq = nc.dram_tensor("wq", [DIM, DSL], F16, kind="ExternalInput")
    wk = nc.dram_tensor("wk", [DIM, DSL], F16, kind="ExternalInput")
    wv = nc.dram_tensor("wv", [DIM, DSL], F16, kind="ExternalInput")
    wo = nc.dram_tensor("wo", [DSL, DIM], F16, kind="ExternalInput")
    emb2 = nc.dram_tensor("emb2", [128, SKV], F16, kind="ExternalInput")
    bq2 = nc.dram_tensor("bq2", [128, 2], F32, kind="ExternalInput")
    bk2 = nc.dram_tensor("bk2", [128, 2], F32, kind="ExternalInput")
    bvr = nc.dram_tensor("bvr", [1, DSL], F16, kind="ExternalInput")
    out = nc.dram_tensor("out", [DIM, SQ], F32, kind="ExternalOutput")

    with tile.TileContext(nc) as tc:
        with tc.tile_pool(name="consts", bufs=1) as consts, \
             tc.tile_pool(name="big", bufs=1) as big, \
             tc.tile_pool(name="xqp", bufs=3) as xqp, \
             tc.tile_pool(name="xap", bufs=12) as xap, \
             tc.tile_pool(name="expp", bufs=32) as expp, \
             tc.tile_pool(name="evp", bufs=6) as evp, \
             tc.tile_pool(name="smallp", bufs=6) as smallp, \
             tc.tile_pool(name="ps512", bufs=2, space="PSUM") as ps512, \
             tc.tile_pool(name="ps1024", bufs=2, space="PSUM") as ps1024, \
             tc.tile_pool(name="psav", bufs=2, space="PSUM") as psav:

            # ---- constants ----
            # only wk (+bk) gate the very first matmuls; the rest of the
            # constants are queued after chunk-0's xa tiles (see kv loop) so
            # the PE starts ~9us earlier
            wk_sb = consts.tile([128, KT, DSL], F16, tag="wk")
            nc.sync.dma_start(out=wk_sb, in_=wk.rearrange("(kt p) m -> p kt m", p=128))
            bk_sb = consts.tile([128, 2], F32, tag="bk")
            nc.sync.dma_start(out=bk_sb, in_=bk2[:, :])
            wv_sb = consts.tile([128, KT, DSL], F16, tag="wv")
            wq_sb = consts.tile([128, KT, DSL], F16, tag="wq")
            emb_sb = consts.tile([128, SKV], F16, tag="emb")
            wo_sb = consts.tile([128, 2, DIM], F16, tag="wo")
            bq_sb = consts.tile([128, 2], F32, tag="bq")
            bv_sb = consts.tile([1, DSL], F16, tag="bv")

            def _late_const_dmas():
                nc.sync.dma_start(out=emb_sb, in_=emb2[:, :])
                nc.sync.dma_start(out=wv_sb, in_=wv.rearrange("(kt p) m -> p kt m", p=128))
                nc.sync.dma_start(out=wq_sb, in_=wq.rearrange("(kt p) m -> p kt m", p=128))
                nc.sync.dma_start(out=bq_sb, in_=bq2[:, :])
                nc.sync.dma_start(out=bv_sb, in_=bvr[:, :])
                nc.sync.dma_start(out=wo_sb, in_=wo.rearrange("(kt p) m -> p kt m", p=128))

            ones_f = consts.tile([1, 128], F32, tag="ones_f")
            nc.vector.memset(ones_f, 1.0)
            ones_h = consts.tile([1, 128], F16, tag="ones_h")
            nc.vector.tensor_copy(ones_h, ones_f)
            onescol_f = consts.tile([128, ST * HPC], F32, tag="onescol")
            nc.vector.memset(onescol_f, 1.0)

            # ---- persistent activations ----
            qT = big.tile([128, 2, SQ], F16, tag="qT")
            kT = big.tile([128, 2, SKV], F16, tag="kT")
            oT0 = big.tile([128, SQ], F16, tag="oT0")
            oT1 = big.tile([128, SQ], F16, tag="oT1")
            oTs = [oT0, oT1]
            v4 = big.tile([128, ST, HPC, 68], F16, tag="v4")
            nc.vector.tensor_copy(
                v4[:, :, :, 64:65],
                onescol_f.rearrange("p (s g) -> p s g", s=ST).unsqueeze(3))

            # ---- phase 1a: k/v projections (full skv needed before attention),
            # with q-proj chunks interleaved to keep the PE fed during the
            # DMA-heavy stretches ----
            exps_store = {}
            next_pair = {}

            def _scores_pair(h, c, p):
                mt, pb = h // 2, (h % 2) * 64
                pss = ps1024.tile([128, 2 * CH], F32, tag="sc",
                                  name=f"pss{h}_{c}_{p}")
                for half in range(2):
                    s2 = 2 * p + half
                    nc.tensor.matmul(
                        pss[:, half * CH:(half + 1) * CH],
                        kT[pb:pb + 64, mt, s2 * 128:(s2 + 1) * 128],
                        qT[pb:pb + 64, mt, c * CH:(c + 1) * CH],
                        start=True, stop=True)
                et = expp.tile([128, 2 * CH], F16, tag="exp", name=f"et{h}_{c}_{p}")
                nc.scalar.activation(et, pss, AF.Exp, scale=SCALE)
                exps_store.setdefault((h, c), []).append(et)
                next_pair[(h, c)] = p + 1

            def _qproj(c):
                psq = [ps512.tile([128, CH], F32, tag="mm", name=f"psq{c}_{i}")
                       for i in range(2)]
                for kt in range(KT):
                    xt = xqp.tile([128, CH], F16, tag="xq")
                    nc.sync.dma_start(
                        out=xt,
                        in_=xq[kt * 128:(kt + 1) * 128, c * CH:(c + 1) * CH])
                    for mt in range(2):
                        nc.tensor.matmul(psq[mt], wq_sb[:, kt, mt * 128:(mt + 1) * 128],
                                         xt, start=(kt == 0), stop=(kt == KT - 1))
                for mt in range(2):
                    nc.vector.tensor_scalar_add(qT[:, mt, c * CH:(c + 1) * CH],
                                                psq[mt], bq_sb[:, mt:mt + 1])

            for c in range(NCH):
                xat = []
                for kt in range(KT):
                    xt = xap.tile([128, CH], F16, tag="xa")
                    nc.sync.dma_start(
                        out=xt,
                        in_=xa[kt * 128:(kt + 1) * 128, c * CH:(c + 1) * CH])
                    xat.append(xt)
                if c == 0:
                    _late_const_dmas()
                psk = [ps512.tile([128, CH], F32, tag="mm", name=f"psk{c}_{i}")
                       for i in range(2)]
                for kt in range(KT):
                    for mt in range(2):
                        nc.tensor.matmul(psk[mt], wk_sb[:, kt, mt * 128:(mt + 1) * 128],
                                         xat[kt], start=(kt == 0), stop=(kt == KT - 1))
                for mt in range(2):
                    # kT = (psum + bk) + emb   (emb rows duplicated across both head halves)
                    nc.vector.scalar_tensor_tensor(
                        kT[:, mt, c * CH:(c + 1) * CH], psk[mt], bk_sb[:, mt:mt + 1],
                        emb_sb[:, c * CH:(c + 1) * CH], ALU.add, ALU.add)
                for j in range(HPC):
                    st = c * HPC + j
                    # v psums use the AV pool (idle during the kv phase) so
                    # early attention scores get the ps1024 slots
                    psv = psav.tile([128, CH], F32, tag="av")
                    for kt in range(KT):
                        nc.tensor.matmul(psv[:, 0:DSL], xat[kt][:, j * 128:(j + 1) * 128],
                                         wv_sb[:, kt, :], start=(kt == 0), stop=False)
                    nc.tensor.matmul(psv[:, 0:DSL], ones_h, bv_sb, start=False, stop=True)
                    nc.vector.tensor_copy(
                        v4[:, st, :, 0:64],
                        psv[:, 0:DSL].rearrange("p (g m) -> p g m", g=HPC))
                _qproj(c)
                # pre-schedule scores+exp for the first heads/chunks so ACT
                # starts working during the kv phase instead of idling
                for (ph, pc) in ((0, 0), (1, 0), (0, 1), (1, 1)):
                    if pc <= c:
                        for p in range(next_pair.get((ph, pc), 0), 2 * (c + 1)):
                            _scores_pair(ph, pc, p)

            # ---- phase 1b+2+3: per sq-chunk: q proj -> attention -> out proj ----
            def _outproj(c):
                for e in range(8):
                    pso = ps512.tile([128, CH], F32, tag="mm", name=f"pso{c}_{e}")
                    for kt in range(2):
                        nc.tensor.matmul(pso, wo_sb[:, kt, e * 128:(e + 1) * 128],
                                         oTs[kt][:, c * CH:(c + 1) * CH],
                                         start=(kt == 0), stop=(kt == 1))
                    ot_sb = evp.tile([128, CH], F32, tag="ev", name=f"ot{c}_{e}")
                    nc.vector.tensor_copy(ot_sb, pso)
                    nc.sync.dma_start(out=out[e * 128:(e + 1) * 128, c * CH:(c + 1) * CH],
                                      in_=ot_sb)

            for c in range(NCH):
                for h in range(HPC):
                    mt, pb = h // 2, (h % 2) * 64
                    for p in range(next_pair.get((h, c), 0), ST // 2):
                        _scores_pair(h, c, p)
                    exps = exps_store[(h, c)]
                    pav = psav.tile([128, CH], F32, tag="av")
                    for s2 in range(ST):
                        nc.tensor.matmul(pav[0:65, :], v4[:, s2, h, 0:65],
                                         exps[s2 // 2][:, (s2 % 2) * CH:(s2 % 2 + 1) * CH],
                                         start=(s2 == 0), stop=(s2 == ST - 1))
                    # exact DVE reciprocal is ~8 cycles/elem; the fast-approx
                    # custom op (~18 bits, 5x faster) is plenty for softmax
                    # denominators in [3e2, 3e5].
                    denrow = smallp.tile([1, CH], F32, tag="rec")
                    nc.vector.tensor_copy(denrow, pav[64:65, :])
                    drec = smallp.tile([1, CH], F32, tag="drec")
                    nc.vector.reciprocal_approx_fast(drec, denrow)
                    bc_sb = smallp.tile([64, CH], F32, tag="bcs")
                    nc.gpsimd.partition_broadcast(bc_sb, drec)
                    nc.vector.tensor_mul(oTs[mt][pb:pb + 64, c * CH:(c + 1) * CH],
                                         pav[0:64, :], bc_sb)

                # out-projection shifted one chunk back: its PE work fills the
                # normalize-chain latency of the current chunk's last head.
                if c > 0:
                    _outproj(c - 1)
            _outproj(NCH - 1)

    nc.compile()
    return nc


def _make_runner(nc):
    """Build a reusable jitted SPMD executor (mirrors bass2jax.run_bass_via_pjrt)."""
    import jax
    import numpy as _np
    from jax.sharding import Mesh, PartitionSpec
    from jax.experimental.shard_map import shard_map
    import concourse.mybir as mybir
    from concourse.bass2jax import (_bass_exec_p, install_neuronx_cc_hook,
                                    partition_id_tensor)

    install_neuronx_cc_hook()
    partition_name = nc.partition_id_tensor.name if nc.partition_id_tensor else None

    in_names, out_names, out_avals, zero_outs = [], [], [], []
    for alloc in nc.m.functions[0].allocations:
        if not isinstance(alloc, mybir.MemoryLocationSet):
            continue
        name = alloc.memorylocations[0].name
        if alloc.kind == "ExternalInput":
            if name != partition_name:
                in_names.append(name)
        elif alloc.kind == "ExternalOutput":
            shape = tuple(alloc.tensor_shape)
            dtype = mybir.dt.np(alloc.dtype)
            out_names.append(name)
            out_avals.append(jax.core.ShapedArray(shape, dtype))
            zero_outs.append(_np.zeros(shape, dtype))
    n_params = len(in_names)
    n_outs = len(out_avals)
    all_in_names = list(in_names) + list(out_names)
    if partition_name is not None:
        all_in_names.append(partition_name)
    donate = tuple(range(n_params, n_params + n_outs))

    def _body(*args):
        operands = list(args)
        if partition_name is not None:
            operands.append(partition_id_tensor())
        outs = _bass_exec_p.bind(
            *operands,
            out_avals=tuple(out_avals),
            in_names=tuple(all_in_names),
            out_names=tuple(out_names),
            lowering_input_output_aliases=(),
            sim_require_finite=True,
            sim_require_nnan=True,
            nc=nc,
        )
        return tuple(outs)

    devices = jax.devices()[:N_CORES]
    mesh = Mesh(np.asarray(devices), ("core",))
    in_specs = (PartitionSpec("core"),) * (n_params + n_outs)
    out_specs = (PartitionSpec("core"),) * n_outs
    sharded = jax.jit(
        shard_map(_body, mesh=mesh, in_specs=in_specs, out_specs=out_specs,
                  check_rep=False),
        donate_argnums=donate, keep_unused=True)
    # non-donating variant for repeat-timing with device-resident operands
    sharded_nd = jax.jit(
        shard_map(_body, mesh=mesh, in_specs=in_specs, out_specs=out_specs,
                  check_rep=False),
        keep_unused=True)

    def _concat(in_maps):
        concat_in = [
            np.concatenate([np.asarray(in_maps[c][name]) for c in range(N_CORES)], axis=0)
            for name in in_names
        ]
        concat_zeros = [np.zeros((N_CORES * z.shape[0], *z.shape[1:]), z.dtype)
                        for z in zero_outs]
        return concat_in, concat_zeros

    def run(in_maps, unpack=True):
        concat_in, concat_zeros = _concat(in_maps)
        out_arrs = sharded(*concat_in, *concat_zeros)
        if not unpack:
            jax.block_until_ready(out_arrs)
            return None
        return [
            {name: np.asarray(out_arrs[i]).reshape(N_CORES, *out_avals[i].shape)[c]
             for i, name in enumerate(out_names)}
            for c in range(N_CORES)
        ]

    def stage(in_maps):
        """device_put all operands once; returns args for timed_call."""
        from jax.sharding import NamedSharding
        sh = NamedSharding(mesh, PartitionSpec("core"))
        concat_in, concat_zeros = _concat(in_maps)
        dev = [jax.device_put(x, sh) for x in concat_in + concat_zeros]
        jax.block_until_ready(dev)
        return dev

    def timed_call(dev_args):
        out_arrs = sharded_nd(*dev_args)
        jax.block_until_ready(out_arrs)
        return out_arrs

    run.stage = stage
    run.timed_call = timed_call
    return run


def _get_runner():
    if "runner" not in _CACHE:
        nc = _build()
        _CACHE["nc"] = nc
        _CACHE["runner"] = _make_runner(nc)
    return _CACHE["runner"]


def _lrope_embT(label_emb, labels):
    inv_freq = (1.0 / (10000.0 ** (np.arange(0, HEAD_DIM, 2, dtype=np.float32)
                                   / HEAD_DIM))).astype(np.float32)
    pos = np.arange(SKV, dtype=np.float32)
    freqs = np.outer(pos, inv_freq)
    emb = np.concatenate([np.sin(freqs), np.cos(freqs)], axis=-1).astype(np.float32)
    lab = np.asarray(label_emb, np.float32)[np.asarray(labels).astype(np.int64)]
    return emb, lab  # [SKV, HD], [B, HD]


def make_in_maps(visual_features, audio_features, audio_labels,
                 Wq, bq, Wk, bk, Wv, bv, Wo, bo, label_emb):
    vis = np.asarray(visual_features, np.float32)
    aud = np.asarray(audio_features, np.float32)
    Wq = np.asarray(Wq, np.float32)
    Wk = np.asarray(Wk, np.float32)
    Wv = np.asarray(Wv, np.float32)
    Wo = np.asarray(Wo, np.float32)
    bq = np.asarray(bq, np.float32)
    bk = np.asarray(bk, np.float32)
    bv = np.asarray(bv, np.float32)
    emb, lab = _lrope_embT(label_emb, audio_labels)

    xqs = [np.ascontiguousarray(vis[b].T).astype(np.float16) for b in range(B)]
    xas = [np.ascontiguousarray(aud[b].T).astype(np.float16) for b in range(B)]
    embs = []
    for b in range(B):
        embT = np.ascontiguousarray((emb * lab[b][None, :]).T)  # [64, SKV]
        embs.append(np.concatenate([embT, embT], axis=0).astype(np.float16))

    in_maps = []
    for core in range(N_CORES):
        b, g = core // HPC, core % HPC
        sl = slice(g * DSL, (g + 1) * DSL)
        in_maps.append({
            "xq": xqs[b],
            "xa": xas[b],
            "wq": np.ascontiguousarray(Wq[sl, :].T).astype(np.float16),
            "wk": np.ascontiguousarray(Wk[sl, :].T).astype(np.float16),
            "wv": np.ascontiguousarray(Wv[sl, :].T).astype(np.float16),
            "wo": np.ascontiguousarray(Wo[:, sl].T).astype(np.float16),
            "emb2": embs[b],
            "bq2": np.ascontiguousarray(bq[sl].reshape(2, 128).T),
            "bk2": np.ascontiguousarray(bk[sl].reshape(2, 128).T),
            "bvr": np.ascontiguousarray(bv[sl].reshape(1, DSL)).astype(np.float16),
        })
    return in_maps


def kernel(**inputs):
    run = _get_runner()
    in_maps = make_in_maps(**inputs)
    results = run(in_maps)
    bo = np.asarray(inputs["bo"], np.float32)
    out = np.empty((B, SQ, DIM), np.float32)
    for b in range(B):
        s = results[4 * b]["out"]
        for g in range(1, HPC):
            s = s + results[4 * b + g]["out"]
        out[b] = s.T + bo[None, :]
    return out


# revision 28
# speedup vs baseline: 1.0137x; 1.0137x over previous
"""AudioCrossAttention on 8 Trainium2 NeuronCores.

Sharding: data-parallel over batch (B=2) x tensor-parallel over heads
(16 heads -> 4 heads / 256 dims per core).  Core c handles batch c//4 and
head-group c%4.  Each core computes its 4 heads' attention plus the partial
output projection over its 256-dim slice; partials are summed on the host
(the unshard step) and bo added there.

Everything on device flows in transposed layout ([d, s] / [skv, sq]) so no
transposes are ever needed:
  qT[d,sq]  = WqT.T @ xT          (lhsT=WqT [din,256], rhs=visual.T)
  kT[d,skv] = WkT.T @ xT (+bk +L-RoPE emb, fused into the PSUM eviction)
  v[skv,d]  = xT.T @ WvT  (natural layout, ones column appended per head)
  scoresT[skv,sq] = kT_h.T @ qT_h          per head, K=hd=64
  expT = exp(0.125 * scoresT)              (no max-subtract; scores are O(5))
  [outT; denom] = [v_h | 1].T @ expT       (ones column -> row 64 = denom)
  outT /= denom  (reciprocal -> gpsimd partition_broadcast -> DVE mult)
  finalT[e,sq] += WoT_c.T @ outT           (partial over this core's d-slice)

Matmul operands are fp16 (cast on host): 1 col/cycle at 2.4 GHz on the PE
vs 4x slower fp32 and 2x slower f32r; accumulation stays fp32 in PSUM, the
final projection partials leave the chip in fp32.
"""

import sys

if '/opt/trn_rl_repo' not in sys.path:
    sys.path.insert(0, '/opt/trn_rl_repo')

import numpy as np

B = 2
SQ = 2048
SKV = 2048
DIM = 1024
NUM_HEADS = 16
HEAD_DIM = 64
N_CORES = 8
HPC = 4          # heads per core
DSL = 256        # d_out slice per core
CH = 512         # sq chunk width
NCH = SQ // CH   # 4
KT = DIM // 128  # 8  d_in k-tiles
ST = SKV // 128  # 16 skv tiles
SCALE = HEAD_DIM ** -0.5

_CACHE = {}


def _build():
    import concourse.bacc as bacc
    import concourse.mybir as mybir
    from concourse import tile

    F32 = mybir.dt.float32
    F16 = mybir.dt.float16
    AF = mybir.ActivationFunctionType
    ALU = mybir.AluOpType

    nc = bacc.Bacc("TRN2", target_bir_lowering=False, debug=False,
                   num_devices=N_CORES)

    xq = nc.dram_tensor("xq", [DIM, SQ], F16, kind="ExternalInput")
    xa = nc.dram_tensor("xa", [DIM, SKV], F16, kind="ExternalInput")
    wq = nc.dram_tensor("wq", [DIM, DSL], F16, kind="ExternalInput")
    wk = nc.dram_tensor("wk", [DIM, DSL], F16, kind="ExternalInput")
    wv = nc.dram_tensor("wv", [DIM, DSL], F16, kind="ExternalInput")
    wo = nc.dram_tensor("wo", [DSL, DIM], F16, kind="ExternalInput")
    emb2 = nc.dram_tensor("emb2", [128, SKV], F16, kind="ExternalInput")
    bq2 = nc.dram_tensor("bq2", [128, 2], F32, kind="ExternalInput")
    bk2 = nc.dram_tensor("bk2", [128, 2], F32, kind="ExternalInput")
    bvr = nc.dram_tensor("bvr", [1, DSL], F16, kind="ExternalInput")
    out = nc.dram_tensor("out", [DIM, SQ], F32, kind="ExternalOutput")

    with tile.TileContext(nc) as tc:
        with tc.tile_pool(name="consts", bufs=1) as consts, \
             tc.tile_pool(name="big", bufs=1) as big, \
             tc.tile_pool(name="xqp", bufs=10) as xqp, \
             tc.tile_pool(name="xap", bufs=12) as xap, \
             tc.tile_pool(name="expp", bufs=32) as expp, \
             tc.tile_pool(name="evp", bufs=6) as evp, \
             tc.tile_pool(name="smallp", bufs=6) as smallp, \
             tc.tile_pool(name="ps512", bufs=1, space="PSUM") as ps512, \
             tc.tile_pool(name="ps1024", bufs=3, space="PSUM") as ps1024, \
             tc.tile_pool(name="psav", bufs=1, space="PSUM") as psav:

            # ---- constants ----
            wk_sb = consts.tile([128, KT, DSL], F16, tag="wk")
            nc.sync.dma_start(out=wk_sb, in_=wk.rearrange("(kt p) m -> p kt m", p=128))
            wv_sb = consts.tile([128, KT, DSL], F16, tag="wv")
            nc.sync.dma_start(out=wv_sb, in_=wv.rearrange("(kt p) m -> p kt m", p=128))
            wq_sb = consts.tile([128, KT, DSL], F16, tag="wq")
            nc.sync.dma_start(out=wq_sb, in_=wq.rearrange("(kt p) m -> p kt m", p=128))
            emb_sb = consts.tile([128, SKV], F16, tag="emb")
            nc.sync.dma_start(out=emb_sb, in_=emb2[:, :])
            wo_sb = consts.tile([128, 2, DIM], F16, tag="wo")
            nc.sync.dma_start(out=wo_sb, in_=wo.rearrange("(kt p) m -> p kt m", p=128))
            bq_sb = consts.tile([128, 2], F32, tag="bq")
            nc.sync.dma_start(out=bq_sb, in_=bq2[:, :])
            bk_sb = consts.tile([128, 2], F32, tag="bk")
            nc.sync.dma_start(out=bk_sb, in_=bk2[:, :])
            bv_sb = consts.tile([1, DSL], F16, tag="bv")
            nc.sync.dma_start(out=bv_sb, in_=bvr[:, :])

            ones_f = consts.tile([1, 128], F32, tag="ones_f")
            nc.vector.memset(ones_f, 1.0)
            ones_h = consts.tile([1, 128], F16, tag="ones_h")
            nc.vector.tensor_copy(ones_h, ones_f)
            onescol_f = consts.tile([128, ST * HPC], F32, tag="onescol")
            nc.vector.memset(onescol_f, 1.0)

            # ---- persistent activations ----
            qT = big.tile([128, 2, SQ], F16, tag="qT")
            kT = big.tile([128, 2, SKV], F16, tag="kT")
            oT0 = big.tile([128, SQ], F16, tag="oT0")
            oT1 = big.tile([128, SQ], F16, tag="oT1")
            oTs = [oT0, oT1]
            v4 = big.tile([128, ST, HPC, 68], F16, tag="v4")
            nc.vector.tensor_copy(
                v4[:, :, :, 64:65],
                onescol_f.rearrange("p (s g) -> p s g", s=ST).unsqueeze(3))

            # ---- phase 1a: k/v projections (full skv needed before attention),
            # with q-proj chunks interleaved to keep the PE fed during the
            # DMA-heavy stretches ----
            exps_store = {}
            next_pair = {}

            def _scores_pair(h, c, p):
                mt, pb = h // 2, (h % 2) * 64
                pss = ps1024.tile([128, 2 * CH], F32, tag="sc",
                                  name=f"pss{h}_{c}_{p}")
                for half in range(2):
                    s2 = 2 * p + half
                    nc.tensor.matmul(
                        pss[:, half * CH:(half + 1) * CH],
                        kT[pb:pb + 64, mt, s2 * 128:(s2 + 1) * 128],
                        qT[pb:pb + 64, mt, c * CH:(c + 1) * CH],
                        start=True, stop=True)
                et = expp.tile([128, 2 * CH], F16, tag="exp", name=f"et{h}_{c}_{p}")
                nc.scalar.activation(et, pss, AF.Exp, scale=SCALE)
                exps_store.setdefault((h, c), []).append(et)
                next_pair[(h, c)] = p + 1

            def _qproj(c):
                xts = []
                for kt in range(KT):
                    xt = xqp.tile([128, CH], F16, tag="xq", name=f"xq{c}_{kt}")
                    nc.sync.dma_start(
                        out=xt,
                        in_=xq[kt * 128:(kt + 1) * 128, c * CH:(c + 1) * CH])
                    xts.append(xt)
                for mt in range(2):
                    psq = ps512.tile([128, CH], F32, tag="mm", name=f"psq{c}_{mt}")
                    for kt in range(KT):
                        nc.tensor.matmul(psq, wq_sb[:, kt, mt * 128:(mt + 1) * 128],
                                         xts[kt], start=(kt == 0), stop=(kt == KT - 1))
                    nc.vector.tensor_scalar_add(qT[:, mt, c * CH:(c + 1) * CH],
                                                psq, bq_sb[:, mt:mt + 1])

            for c in range(NCH):
                xat = []
                for kt in range(KT):
                    xt = xap.tile([128, CH], F16, tag="xa")
                    nc.sync.dma_start(
                        out=xt,
                        in_=xa[kt * 128:(kt + 1) * 128, c * CH:(c + 1) * CH])
                    xat.append(xt)
                for mt in range(2):
                    psk = ps512.tile([128, CH], F32, tag="mm", name=f"psk{c}_{mt}")
                    for kt in range(KT):
                        nc.tensor.matmul(psk, wk_sb[:, kt, mt * 128:(mt + 1) * 128],
                                         xat[kt], start=(kt == 0), stop=(kt == KT - 1))
                    # kT = (psum + bk) + emb   (emb rows duplicated across both head halves)
                    nc.vector.scalar_tensor_tensor(
                        kT[:, mt, c * CH:(c + 1) * CH], psk, bk_sb[:, mt:mt + 1],
                        emb_sb[:, c * CH:(c + 1) * CH], ALU.add, ALU.add)
                for j in range(HPC):
                    st = c * HPC + j
                    # v psums use the AV pool (idle during the kv phase) so
                    # early attention scores get the ps1024 slots
                    psv = psav.tile([128, CH], F32, tag="av")
                    for kt in range(KT):
                        nc.tensor.matmul(psv[:, 0:DSL], xat[kt][:, j * 128:(j + 1) * 128],
                                         wv_sb[:, kt, :], start=(kt == 0), stop=False)
                    nc.tensor.matmul(psv[:, 0:DSL], ones_h, bv_sb, start=False, stop=True)
                    nc.vector.tensor_copy(
                        v4[:, st, :, 0:64],
                        psv[:, 0:DSL].rearrange("p (g m) -> p g m", g=HPC))
                _qproj(c)
                # pre-schedule scores+exp for the first heads/chunks so ACT
                # starts working during the kv phase instead of idling
                for (ph, pc) in ((0, 0), (1, 0), (0, 1), (1, 1)):
                    if pc <= c:
                        for p in range(next_pair.get((ph, pc), 0), 2 * (c + 1)):
                            _scores_pair(ph, pc, p)

            # ---- phase 1b+2+3: per sq-chunk: q proj -> attention -> out proj ----
            def _outproj(c):
                for e in range(8):
                    pso = ps512.tile([128, CH], F32, tag="mm", name=f"pso{c}_{e}")
                    for kt in range(2):
                        nc.tensor.matmul(pso, wo_sb[:, kt, e * 128:(e + 1) * 128],
                                         oTs[kt][:, c * CH:(c + 1) * CH],
                                         start=(kt == 0), stop=(kt == 1))
                    ot_sb = evp.tile([128, CH], F32, tag="ev", name=f"ot{c}_{e}")
                    nc.vector.tensor_copy(ot_sb, pso)
                    nc.sync.dma_start(out=out[e * 128:(e + 1) * 128, c * CH:(c + 1) * CH],
                                      in_=ot_sb)

            for c in range(NCH):
                for h in range(HPC):
                    mt, pb = h // 2, (h % 2) * 64
                    for p in range(next_pair.get((h, c), 0), ST // 2):
                        _scores_pair(h, c, p)
                    exps = exps_store[(h, c)]
                    pav = psav.tile([128, CH], F32, tag="av")
                    for s2 in range(ST):
                        nc.tensor.matmul(pav[0:65, :], v4[:, s2, h, 0:65],
                                         exps[s2 // 2][:, (s2 % 2) * CH:(s2 % 2 + 1) * CH],
                                         start=(s2 == 0), stop=(s2 == ST - 1))
                    # exact DVE reciprocal is ~8 cycles/elem; the fast-approx
                    # custom op (~18 bits, 5x faster) is plenty for softmax
                    # denominators in [3e2, 3e5].
                    denrow = smallp.tile([1, CH], F32, tag="rec")
                    nc.vector.tensor_copy(denrow, pav[64:65, :])
                    drec = smallp.tile([1, CH], F32, tag="drec")
                    nc.vector.reciprocal_approx_fast(drec, denrow)
                    bc_sb = smallp.tile([64, CH], F32, tag="bcs")
                    nc.gpsimd.partition_broadcast(bc_sb, drec)
                    nc.vector.tensor_mul(oTs[mt][pb:pb + 64, c * CH:(c + 1) * CH],
                                         pav[0:64, :], bc_sb)

                # out-projection shifted one chunk back: its PE work fills the
                # normalize-chain latency of the current chunk's last head.
                if c > 0:
                    _outproj(c - 1)
            _outproj(NCH - 1)

    nc.compile()
    return nc


def _make_runner(nc):
    """Build a reusable jitted SPMD executor (mirrors bass2jax.run_bass_via_pjrt)."""
    import jax
    import numpy as _np
    from jax.sharding import Mesh, PartitionSpec
    from jax.experimental.shard_map import shard_map
    import concourse.mybir as mybir
    from concourse.bass2jax import (_bass_exec_p, install_neuronx_cc_hook,
                                    partition_id_tensor)

    install_neuronx_cc_hook()
    partition_name = nc.partition_id_tensor.name if nc.partition_id_tensor else None

    in_names, out_names, out_avals, zero_outs = [], [], [], []
    for alloc in nc.m.functions[0].allocations:
        if not isinstance(alloc, mybir.MemoryLocationSet):
            continue
        name = alloc.memorylocations[0].name
        if alloc.kind == "ExternalInput":
            if name != partition_name:
                in_names.append(name)
        elif alloc.kind == "ExternalOutput":
            shape = tuple(alloc.tensor_shape)
            dtype = mybir.dt.np(alloc.dtype)
            out_names.append(name)
            out_avals.append(jax.core.ShapedArray(shape, dtype))
            zero_outs.append(_np.zeros(shape, dtype))
    n_params = len(in_names)
    n_outs = len(out_avals)
    all_in_names = list(in_names) + list(out_names)
    if partition_name is not None:
        all_in_names.append(partition_name)
    donate = tuple(range(n_params, n_params + n_outs))

    def _body(*args):
        operands = list(args)
        if partition_name is not None:
            operands.append(partition_id_tensor())
        outs = _bass_exec_p.bind(
            *operands,
            out_avals=tuple(out_avals),
            in_names=tuple(all_in_names),
            out_names=tuple(out_names),
            lowering_input_output_aliases=(),
            sim_require_finite=True,
            sim_require_nnan=True,
            nc=nc,
        )
        return tuple(outs)

    devices = jax.devices()[:N_CORES]
    mesh = Mesh(np.asarray(devices), ("core",))
    in_specs = (PartitionSpec("core"),) * (n_params + n_outs)
    out_specs = (PartitionSpec("core"),) * n_outs
    sharded = jax.jit(
        shard_map(_body, mesh=mesh, in_specs=in_specs, out_specs=out_specs,
                  check_rep=False),
        donate_argnums=donate, keep_unused=True)
    # non-donating variant for repeat-timing with device-resident operands
    sharded_nd = jax.jit(
        shard_map(_body, mesh=mesh, in_specs=in_specs, out_specs=out_specs,
                  check_rep=False),
        keep_unused=True)

    def _concat(in_maps):
        concat_in = [
            np.concatenate([np.asarray(in_maps[c][name]) for c in range(N_CORES)], axis=0)
            for name in in_names
        ]
        concat_zeros = [np.zeros((N_CORES * z.shape[0], *z.shape[1:]), z.dtype)
                        for z in zero_outs]
        return concat_in, concat_zeros

    def run(in_maps, unpack=True):
        concat_in, concat_zeros = _concat(in_maps)
        out_arrs = sharded(*concat_in, *concat_zeros)
        if not unpack:
            jax.block_until_ready(out_arrs)
            return None
        return [
            {name: np.asarray(out_arrs[i]).reshape(N_CORES, *out_avals[i].shape)[c]
             for i, name in enumerate(out_names)}
            for c in range(N_CORES)
        ]

    def stage(in_maps):
        """device_put all operands once; returns args for timed_call."""
        from jax.sharding import NamedSharding
        sh = NamedSharding(mesh, PartitionSpec("core"))
        concat_in, concat_zeros = _concat(in_maps)
        dev = [jax.device_put(x, sh) for x in concat_in + concat_zeros]
        jax.block_until_ready(dev)
        return dev

    def timed_call(dev_args):
        out_arrs = sharded_nd(*dev_args)
        jax.block_until_ready(out_arrs)
        return out_arrs

    run.stage = stage
    run.timed_call = timed_call
    return run


def _get_runner():
    if "runner" not in _CACHE:
        nc = _build()
        _CACHE["nc"] = nc
        _CACHE["runner"] = _make_runner(nc)
    return _CACHE["runner"]


def _lrope_embT(label_emb, labels):
    inv_freq = (1.0 / (10000.0 ** (np.arange(0, HEAD_DIM, 2, dtype=np.float32)
                                   / HEAD_DIM))).astype(np.float32)
    pos = np.arange(SKV, dtype=np.float32)
    freqs = np.outer(pos, inv_freq)
    emb = np.concatenate([np.sin(freqs), np.cos(freqs)], axis=-1).astype(np.float32)
    lab = np.asarray(label_emb, np.float32)[np.asarray(labels).astype(np.int64)]
    return emb, lab  # [SKV, HD], [B, HD]


def make_in_maps(visual_features, audio_features, audio_labels,
                 Wq, bq, Wk, bk, Wv, bv, Wo, bo, label_emb):
    vis = np.asarray(visual_features, np.float32)
    aud = np.asarray(audio_features, np.float32)
    Wq = np.asarray(Wq, np.float32)
    Wk = np.asarray(Wk, np.float32)
    Wv = np.asarray(Wv, np.float32)
    Wo = np.asarray(Wo, np.float32)
    bq = np.asarray(bq, np.float32)
    bk = np.asarray(bk, np.float32)
    bv = np.asarray(bv, np.float32)
    emb, lab = _lrope_embT(label_emb, audio_labels)

    xqs = [np.ascontiguousarray(vis[b].T).astype(np.float16) for b in range(B)]
    xas = [np.ascontiguousarray(aud[b].T).astype(np.float16) for b in range(B)]
    embs = []
    for b in range(B):
        embT = np.ascontiguousarray((emb * lab[b][None, :]).T)  # [64, SKV]
        embs.append(np.concatenate([embT, embT], axis=0).astype(np.float16))

    in_maps = []
    for core in range(N_CORES):
        b, g = core // HPC, core % HPC
        sl = slice(g * DSL, (g + 1) * DSL)
        in_maps.append({
            "xq": xqs[b],
            "xa": xas[b],
            "wq": np.ascontiguousarray(Wq[sl, :].T).astype(np.float16),
            "wk": np.ascontiguousarray(Wk[sl, :].T).astype(np.float16),
            "wv": np.ascontiguousarray(Wv[sl, :].T).astype(np.float16),
            "wo": np.ascontiguousarray(Wo[:, sl].T).astype(np.float16),
            "emb2": embs[b],
            "bq2": np.ascontiguousarray(bq[sl].reshape(2, 128).T),
            "bk2": np.ascontiguousarray(bk[sl].reshape(2, 128).T),
            "bvr": np.ascontiguousarray(bv[sl].reshape(1, DSL)).astype(np.float16),
        })
    return in_maps


def kernel(**inputs):
    run = _get_runner()
    in_maps = make_in_maps(**inputs)
    results = run(in_maps)
    bo = np.asarray(inputs["bo"], np.float32)
    out = np.empty((B, SQ, DIM), np.float32)
    for b in range(B):
        s = results[4 * b]["out"]
        for g in range(1, HPC):
            s = s + results[4 * b + g]["out"]
        out[b] = s.T + bo[None, :]
    return out


# revision 29
# speedup vs baseline: 1.0196x; 1.0058x over previous
"""AudioCrossAttention on 8 Trainium2 NeuronCores.

Sharding: data-parallel over batch (B=2) x tensor-parallel over heads
(16 heads -> 4 heads / 256 dims per core).  Core c handles batch c//4 and
head-group c%4.  Each core computes its 4 heads' attention plus the partial
output projection over its 256-dim slice; partials are summed on the host
(the unshard step) and bo added there.

Everything on device flows in transposed layout ([d, s] / [skv, sq]) so no
transposes are ever needed:
  qT[d,sq]  = WqT.T @ xT          (lhsT=WqT [din,256], rhs=visual.T)
  kT[d,skv] = WkT.T @ xT (+bk +L-RoPE emb, fused into the PSUM eviction)
  v[skv,d]  = xT.T @ WvT  (natural layout, ones column appended per head)
  scoresT[skv,sq] = kT_h.T @ qT_h          per head, K=hd=64
  expT = exp(0.125 * scoresT)              (no max-subtract; scores are O(5))
  [outT; denom] = [v_h | 1].T @ expT       (ones column -> row 64 = denom)
  outT /= denom  (reciprocal -> gpsimd partition_broadcast -> DVE mult)
  finalT[e,sq] += WoT_c.T @ outT           (partial over this core's d-slice)

Matmul operands are fp16 (cast on host): 1 col/cycle at 2.4 GHz on the PE
vs 4x slower fp32 and 2x slower f32r; accumulation stays fp32 in PSUM, the
final projection partials leave the chip in fp32.
"""

import sys

if '/opt/trn_rl_repo' not in sys.path:
    sys.path.insert(0, '/opt/trn_rl_repo')

import numpy as np

B = 2
SQ = 2048
SKV = 2048
DIM = 1024
NUM_HEADS = 16
HEAD_DIM = 64
N_CORES = 8
HPC = 4          # heads per core
DSL = 256        # d_out slice per core
CH = 512         # sq chunk width
NCH = SQ // CH   # 4
KT = DIM // 128  # 8  d_in k-tiles
ST = SKV // 128  # 16 skv tiles
SCALE = HEAD_DIM ** -0.5

_CACHE = {}


def _build():
    import concourse.bacc as bacc
    import concourse.mybir as mybir
    from concourse import tile

    F32 = mybir.dt.float32
    F16 = mybir.dt.float16
    AF = mybir.ActivationFunctionType
    ALU = mybir.AluOpType

    nc = bacc.Bacc("TRN2", target_bir_lowering=False, debug=False,
                   num_devices=N_CORES)

    xq = nc.dram_tensor("xq", [DIM, SQ], F16, kind="ExternalInput")
    xa = nc.dram_tensor("xa", [DIM, SKV], F16, kind="ExternalInput")
    wq = nc.dram_tensor("wq", [DIM, DSL], F16, kind="ExternalInput")
    wk = nc.dram_tensor("wk", [DIM, DSL], F16, kind="ExternalInput")
    wv = nc.dram_tensor("wv", [DIM, DSL], F16, kind="ExternalInput")
    wo = nc.dram_tensor("wo", [DSL, DIM], F16, kind="ExternalInput")
    emb2 = nc.dram_tensor("emb2", [128, SKV], F16, kind="ExternalInput")
    bq2 = nc.dram_tensor("bq2", [128, 2], F32, kind="ExternalInput")
    bk2 = nc.dram_tensor("bk2", [128, 2], F32, kind="ExternalInput")
    bvr = nc.dram_tensor("bvr", [1, DSL], F16, kind="ExternalInput")
    out = nc.dram_tensor("out", [DIM, SQ], F32, kind="ExternalOutput")

    with tile.TileContext(nc) as tc:
        with tc.tile_pool(name="consts", bufs=1) as consts, \
             tc.tile_pool(name="big", bufs=1) as big, \
             tc.tile_pool(name="xqp", bufs=3) as xqp, \
             tc.tile_pool(name="xap", bufs=12) as xap, \
             tc.tile_pool(name="expp", bufs=32) as expp, \
             tc.tile_pool(name="evp", bufs=6) as evp, \
             tc.tile_pool(name="smallp", bufs=6) as smallp, \
             tc.tile_pool(name="ps512", bufs=2, space="PSUM") as ps512, \
             tc.tile_pool(name="ps1024", bufs=2, space="PSUM") as ps1024, \
             tc.tile_pool(name="psav", bufs=2, space="PSUM") as psav:

            # ---- constants ----
            wk_sb = consts.tile([128, KT, DSL], F16, tag="wk")
            nc.sync.dma_start(out=wk_sb, in_=wk.rearrange("(kt p) m -> p kt m", p=128))
            wv_sb = consts.tile([128, KT, DSL], F16, tag="wv")
            nc.sync.dma_start(out=wv_sb, in_=wv.rearrange("(kt p) m -> p kt m", p=128))
            wq_sb = consts.tile([128, KT, DSL], F16, tag="wq")
            nc.sync.dma_start(out=wq_sb, in_=wq.rearrange("(kt p) m -> p kt m", p=128))
            emb_sb = consts.tile([128, SKV], F16, tag="emb")
            nc.sync.dma_start(out=emb_sb, in_=emb2[:, :])
            wo_sb = consts.tile([128, 2, DIM], F16, tag="wo")
            nc.sync.dma_start(out=wo_sb, in_=wo.rearrange("(kt p) m -> p kt m", p=128))
            bq_sb = consts.tile([128, 2], F32, tag="bq")
            nc.sync.dma_start(out=bq_sb, in_=bq2[:, :])
            bk_sb = consts.tile([128, 2], F32, tag="bk")
            nc.sync.dma_start(out=bk_sb, in_=bk2[:, :])
            bv_sb = consts.tile([1, DSL], F16, tag="bv")
            nc.sync.dma_start(out=bv_sb, in_=bvr[:, :])

            ones_f = consts.tile([1, 128], F32, tag="ones_f")
            nc.vector.memset(ones_f, 1.0)
            ones_h = consts.tile([1, 128], F16, tag="ones_h")
            nc.vector.tensor_copy(ones_h, ones_f)
            onescol_f = consts.tile([128, ST * HPC], F32, tag="onescol")
            nc.vector.memset(onescol_f, 1.0)

            # ---- persistent activations ----
            qT = big.tile([128, 2, SQ], F16, tag="qT")
            kT = big.tile([128, 2, SKV], F16, tag="kT")
            oT0 = big.tile([128, SQ], F16, tag="oT0")
            oT1 = big.tile([128, SQ], F16, tag="oT1")
            oTs = [oT0, oT1]
            v4 = big.tile([128, ST, HPC, 68], F16, tag="v4")
            nc.vector.tensor_copy(
                v4[:, :, :, 64:65],
                onescol_f.rearrange("p (s g) -> p s g", s=ST).unsqueeze(3))

            # ---- phase 1a: k/v projections (full skv needed before attention),
            # with q-proj chunks interleaved to keep the PE fed during the
            # DMA-heavy stretches ----
            exps_store = {}
            next_pair = {}

            def _scores_pair(h, c, p):
                mt, pb = h // 2, (h % 2) * 64
                pss = ps1024.tile([128, 2 * CH], F32, tag="sc",
                                  name=f"pss{h}_{c}_{p}")
                for half in range(2):
                    s2 = 2 * p + half
                    nc.tensor.matmul(
                        pss[:, half * CH:(half + 1) * CH],
                        kT[pb:pb + 64, mt, s2 * 128:(s2 + 1) * 128],
                        qT[pb:pb + 64, mt, c * CH:(c + 1) * CH],
                        start=True, stop=True)
                et = expp.tile([128, 2 * CH], F16, tag="exp", name=f"et{h}_{c}_{p}")
                nc.scalar.activation(et, pss, AF.Exp, scale=SCALE)
                exps_store.setdefault((h, c), []).append(et)
                next_pair[(h, c)] = p + 1

            def _qproj(c):
                psq = [ps512.tile([128, CH], F32, tag="mm", name=f"psq{c}_{i}")
                       for i in range(2)]
                for kt in range(KT):
                    xt = xqp.tile([128, CH], F16, tag="xq")
                    nc.sync.dma_start(
                        out=xt,
                        in_=xq[kt * 128:(kt + 1) * 128, c * CH:(c + 1) * CH])
                    for mt in range(2):
                        nc.tensor.matmul(psq[mt], wq_sb[:, kt, mt * 128:(mt + 1) * 128],
                                         xt, start=(kt == 0), stop=(kt == KT - 1))
                for mt in range(2):
                    nc.vector.tensor_scalar_add(qT[:, mt, c * CH:(c + 1) * CH],
                                                psq[mt], bq_sb[:, mt:mt + 1])

            for c in range(NCH):
                xat = []
                for kt in range(KT):
                    xt = xap.tile([128, CH], F16, tag="xa")
                    nc.sync.dma_start(
                        out=xt,
                        in_=xa[kt * 128:(kt + 1) * 128, c * CH:(c + 1) * CH])
                    xat.append(xt)
                psk = [ps512.tile([128, CH], F32, tag="mm", name=f"psk{c}_{i}")
                       for i in range(2)]
                for kt in range(KT):
                    for mt in range(2):
                        nc.tensor.matmul(psk[mt], wk_sb[:, kt, mt * 128:(mt + 1) * 128],
                                         xat[kt], start=(kt == 0), stop=(kt == KT - 1))
                for mt in range(2):
                    # kT = (psum + bk) + emb   (emb rows duplicated across both head halves)
                    nc.vector.scalar_tensor_tensor(
                        kT[:, mt, c * CH:(c + 1) * CH], psk[mt], bk_sb[:, mt:mt + 1],
                        emb_sb[:, c * CH:(c + 1) * CH], ALU.add, ALU.add)
                for j in range(HPC):
                    st = c * HPC + j
                    # v psums use the AV pool (idle during the kv phase) so
                    # early attention scores get the ps1024 slots
                    psv = psav.tile([128, CH], F32, tag="av")
                    for kt in range(KT):
                        nc.tensor.matmul(psv[:, 0:DSL], xat[kt][:, j * 128:(j + 1) * 128],
                                         wv_sb[:, kt, :], start=(kt == 0), stop=False)
                    nc.tensor.matmul(psv[:, 0:DSL], ones_h, bv_sb, start=False, stop=True)
                    nc.vector.tensor_copy(
                        v4[:, st, :, 0:64],
                        psv[:, 0:DSL].rearrange("p (g m) -> p g m", g=HPC))
                _qproj(c)
                # pre-schedule scores+exp for the first heads/chunks so ACT
                # starts working during the kv phase instead of idling
                for (ph, pc) in ((0, 0), (1, 0), (0, 1), (1, 1)):
                    if pc <= c:
                        for p in range(next_pair.get((ph, pc), 0), 2 * (c + 1)):
                            _scores_pair(ph, pc, p)

            # ---- phase 1b+2+3: per sq-chunk: q proj -> attention -> out proj ----
            def _outproj(c):
                for e in range(8):
                    pso = ps512.tile([128, CH], F32, tag="mm", name=f"pso{c}_{e}")
                    for kt in range(2):
                        nc.tensor.matmul(pso, wo_sb[:, kt, e * 128:(e + 1) * 128],
                                         oTs[kt][:, c * CH:(c + 1) * CH],
                                         start=(kt == 0), stop=(kt == 1))
                    ot_sb = evp.tile([128, CH], F32, tag="ev", name=f"ot{c}_{e}")
                    nc.vector.tensor_copy(ot_sb, pso)
                    nc.sync.dma_start(out=out[e * 128:(e + 1) * 128, c * CH:(c + 1) * CH],
                                      in_=ot_sb)

            for c in range(NCH):
                for h in range(HPC):
                    mt, pb = h // 2, (h % 2) * 64
                    for p in range(next_pair.get((h, c), 0), ST // 2):
                        _scores_pair(h, c, p)
                    exps = exps_store[(h, c)]
                    pav = psav.tile([128, CH], F32, tag="av")
                    for s2 in range(ST):
                        nc.tensor.matmul(pav[0:65, :], v4[:, s2, h, 0:65],
                                         exps[s2 // 2][:, (s2 % 2) * CH:(s2 % 2 + 1) * CH],
                                         start=(s2 == 0), stop=(s2 == ST - 1))
                    # exact DVE reciprocal is ~8 cycles/elem; the fast-approx
                    # custom op (~18 bits, 5x faster) is plenty for softmax
                    # denominators in [3e2, 3e5].
                    denrow = smallp.tile([1, CH], F32, tag="rec")
                    nc.vector.tensor_copy(denrow, pav[64:65, :])
                    drec = smallp.tile([1, CH], F32, tag="drec")
                    nc.vector.reciprocal_approx_fast(drec, denrow)
                    bc_sb = smallp.tile([64, CH], F32, tag="bcs")
                    nc.gpsimd.partition_broadcast(bc_sb, drec)
                    nc.vector.tensor_mul(oTs[mt][pb:pb + 64, c * CH:(c + 1) * CH],
                                         pav[0:64, :], bc_sb)

                # out-projection shifted one chunk back: its PE work fills the
                # normalize-chain latency of the current chunk's last head.
                if c > 0:
                    _outproj(c - 1)
            _outproj(NCH - 1)

    nc.compile()
    return nc


def _make_runner(nc):
    """Build a reusable jitted SPMD executor (mirrors bass2jax.run_bass_via_pjrt)."""
    import jax
    import numpy as _np
    from jax.sharding import Mesh, PartitionSpec
    from jax.experimental.shard_map import shard_map
    import concourse.mybir as mybir
    from concourse.bass2jax import (_bass_exec_p, install_neuronx_cc_hook,
                                    partition_id_tensor)

    install_neuronx_cc_hook()
    partition_name = nc.partition_id_tensor.name if nc.partition_id_tensor else None

    in_names, out_names, out_avals, zero_outs = [], [], [], []
    for alloc in nc.m.functions[0].allocations:
        if not isinstance(alloc, mybir.MemoryLocationSet):
            continue
        name = alloc.memorylocations[0].name
        if alloc.kind == "ExternalInput":
            if name != partition_name:
                in_names.append(name)
        elif alloc.kind == "ExternalOutput":
            shape = tuple(alloc.tensor_shape)
            dtype = mybir.dt.np(alloc.dtype)
            out_names.append(name)
            out_avals.append(jax.core.ShapedArray(shape, dtype))
            zero_outs.append(_np.zeros(shape, dtype))
    n_params = len(in_names)
    n_outs = len(out_avals)
    all_in_names = list(in_names) + list(out_names)
    if partition_name is not None:
        all_in_names.append(partition_name)
    donate = tuple(range(n_params, n_params + n_outs))

    def _body(*args):
        operands = list(args)
        if partition_name is not None:
            operands.append(partition_id_tensor())
        outs = _bass_exec_p.bind(
            *operands,
            out_avals=tuple(out_avals),
            in_names=tuple(all_in_names),
            out_names=tuple(out_names),
            lowering_input_output_aliases=(),
            sim_require_finite=True,
            sim_require_nnan=True,
            nc=nc,
        )
        return tuple(outs)

    devices = jax.devices()[:N_CORES]
    mesh = Mesh(np.asarray(devices), ("core",))
    in_specs = (PartitionSpec("core"),) * (n_params + n_outs)
    out_specs = (PartitionSpec("core"),) * n_outs
    sharded = jax.jit(
        shard_map(_body, mesh=mesh, in_specs=in_specs, out_specs=out_specs,
                  check_rep=False),
        donate_argnums=donate, keep_unused=True)
    # non-donating variant for repeat-timing with device-resident operands
    sharded_nd = jax.jit(
        shard_map(_body, mesh=mesh, in_specs=in_specs, out_specs=out_specs,
                  check_rep=False),
        keep_unused=True)

    def _concat(in_maps):
        concat_in = [
            np.concatenate([np.asarray(in_maps[c][name]) for c in range(N_CORES)], axis=0)
            for name in in_names
        ]
        concat_zeros = [np.zeros((N_CORES * z.shape[0], *z.shape[1:]), z.dtype)
                        for z in zero_outs]
        return concat_in, concat_zeros

    def run(in_maps, unpack=True):
        concat_in, concat_zeros = _concat(in_maps)
        out_arrs = sharded(*concat_in, *concat_zeros)
        if not unpack:
            jax.block_until_ready(out_arrs)
            return None
        return [
            {name: np.asarray(out_arrs[i]).reshape(N_CORES, *out_avals[i].shape)[c]
             for i, name in enumerate(out_names)}
            for c in range(N_CORES)
        ]

    def stage(in_maps):
        """device_put all operands once; returns args for timed_call."""
        from jax.sharding import NamedSharding
        sh = NamedSharding(mesh, PartitionSpec("core"))
        concat_in, concat_zeros = _concat(in_maps)
        dev = [jax.device_put(x, sh) for x in concat_in + concat_zeros]
        jax.block_until_ready(dev)
        return dev

    def timed_call(dev_args):
        out_arrs = sharded_nd(*dev_args)
        jax.block_until_ready(out_arrs)
        return out_arrs

    run.stage = stage
    run.timed_call = timed_call
    return run


def _get_runner():
    if "runner" not in _CACHE:
        nc = _build()
        _CACHE["nc"] = nc
        _CACHE["runner"] = _make_runner(nc)
    return _CACHE["runner"]


def _lrope_embT(label_emb, labels):
    inv_freq = (1.0 / (10000.0 ** (np.arange(0, HEAD_DIM, 2, dtype=np.float32)
                                   / HEAD_DIM))).astype(np.float32)
    pos = np.arange(SKV, dtype=np.float32)
    freqs = np.outer(pos, inv_freq)
    emb = np.concatenate([np.sin(freqs), np.cos(freqs)], axis=-1).astype(np.float32)
    lab = np.asarray(label_emb, np.float32)[np.asarray(labels).astype(np.int64)]
    return emb, lab  # [SKV, HD], [B, HD]


def make_in_maps(visual_features, audio_features, audio_labels,
                 Wq, bq, Wk, bk, Wv, bv, Wo, bo, label_emb):
    vis = np.asarray(visual_features, np.float32)
    aud = np.asarray(audio_features, np.float32)
    Wq = np.asarray(Wq, np.float32)
    Wk = np.asarray(Wk, np.float32)
    Wv = np.asarray(Wv, np.float32)
    Wo = np.asarray(Wo, np.float32)
    bq = np.asarray(bq, np.float32)
    bk = np.asarray(bk, np.float32)
    bv = np.asarray(bv, np.float32)
    emb, lab = _lrope_embT(label_emb, audio_labels)

    xqs = [np.ascontiguousarray(vis[b].T).astype(np.float16) for b in range(B)]
    xas = [np.ascontiguousarray(aud[b].T).astype(np.float16) for b in range(B)]
    embs = []
    for b in range(B):
        embT = np.ascontiguousarray((emb * lab[b][None, :]).T)  # [64, SKV]
        embs.append(np.concatenate([embT, embT], axis=0).astype(np.float16))

    in_maps = []
    for core in range(N_CORES):
        b, g = core // HPC, core % HPC
        sl = slice(g * DSL, (g + 1) * DSL)
        in_maps.append({
            "xq": xqs[b],
            "xa": xas[b],
            "wq": np.ascontiguousarray(Wq[sl, :].T).astype(np.float16),
            "wk": np.ascontiguousarray(Wk[sl, :].T).astype(np.float16),
            "wv": np.ascontiguousarray(Wv[sl, :].T).astype(np.float16),
            "wo": np.ascontiguousarray(Wo[:, sl].T).astype(np.float16),
            "emb2": embs[b],
            "bq2": np.ascontiguousarray(bq[sl].reshape(2, 128).T),
            "bk2": np.ascontiguousarray(bk[sl].reshape(2, 128).T),
            "bvr": np.ascontiguousarray(bv[sl].reshape(1, DSL)).astype(np.float16),
        })
    return in_maps


def kernel(**inputs):
    run = _get_runner()
    in_maps = make_in_maps(**inputs)
    results = run(in_maps)
    bo = np.asarray(inputs["bo"], np.float32)
    out = np.empty((B, SQ, DIM), np.float32)
    for b in range(B):
        s = results[4 * b]["out"]
        for g in range(1, HPC):
            s = s + results[4 * b + g]["out"]
        out[b] = s.T + bo[None, :]
    return out
